# revision 21
# baseline (speedup 1.0000x reference)
"""Trainium2 Bass kernel for nn_MultiHeadAttention_70549132804637.

Reference computation (note: NO softmax — values use raw scaled logits):
    qkv = x @ w_qkv.T                         (B,S,3E) split per head into q,k,v
    logits = (q @ k^T) * scale                (B,H,S,S)
    values = logits @ v                       (B,H,S,D)
    out    = concat_heads(values) @ w_o.T     (B,S,E)

Because there is no softmax, (q k^T) v == q (k^T v): we compute the tiny
64x64 per-head matrix M = scale * (k^T v) and then values = q @ M, which
drops the attention cost from O(S^2 D) to O(S D^2) per head.

Sharding over 8 cores: core c handles batch b = c//2 and head-half
hh = c%2 (8 of 16 heads).  Each core computes a partial output
(its heads' contribution through w_o); the host sums the two partials
per batch.  All matmul operands are bf16 (PE upconverts to FP22 and
accumulates FP32 in PSUM; bf16 weights get FWL fast weight-loads, which
hides the per-matmul LDWEIGHTS cost that fp32/fp32r would expose).  A
short stream of dummy matmuls during the DMA lead-in keeps the PE's HAM
activity monitor busy so real matmuls start at the full 2.4 GHz clock.

Per-core dataflow (all tokens T=2048, E=1024, 8 local heads, D=64):
  phase A: qT[j, t]  = wq^T-slices (stationary) x xT (moving)     j=512 rows
  phase B: kv[t, j]  = xT-slices (stationary) x wkv (moving)      j=1024 cols
           M_pair    += k_pair^T @ v-window  (PSUM-resident, 16 t-tiles)
  phase C: msb_pair  = blockdiag(scale*M_h0, scale*M_h1) in SBUF
           vT[f, t]  = msb_pair (stationary) x qT (moving)
  phase D: out[t, u] = vT-slices (stationary) x wo (moving), acc over f
"""

from contextlib import ExitStack

import numpy as np

import concourse.mybir as mybir
import concourse.tile as tile
from concourse import bacc
from concourse.bass import ts
from concourse.bass_utils import run_bass_kernel_spmd

F32 = mybir.dt.float32
BF16 = mybir.dt.bfloat16

B, S, E, H = 4, 2048, 1024, 16
D = E // H                 # 64 per-head dim
SCALE = 0.125              # 1/sqrt(D), exact power of two
NCORES = 8
HPC = H // 2               # heads per core = 8
P = 128
ET = E // P                # 8 e-tiles (contraction tiles for projections)
TC = 4                     # token chunks
TW = S // TC               # 512 tokens per chunk
TT = S // P                # 16 token tiles
NPAIR = HPC // 2           # 4 head pairs per core
JQ = HPC * D               # 512 q columns per core

_MODULE = None


def _voff(p):
    # 256-wide window into the v block covering the pair's 128 columns
    return min(128 * p, 256)


def _emit(tc_, nc, xT, wq, wkv, wo, out):
    with ExitStack() as ctx:
        xp = ctx.enter_context(tc_.tile_pool(name="xp", bufs=ET))
        wqp = ctx.enter_context(tc_.tile_pool(name="wqp", bufs=ET))
        wkvp = ctx.enter_context(tc_.tile_pool(name="wkvp", bufs=ET))
        wop = ctx.enter_context(tc_.tile_pool(name="wop", bufs=NPAIR))
        qtp = ctx.enter_context(tc_.tile_pool(name="qtp", bufs=NPAIR))
        kvp = ctx.enter_context(tc_.tile_pool(name="kvp", bufs=4))
        msbp = ctx.enter_context(tc_.tile_pool(name="msbp", bufs=NPAIR))
        vtp = ctx.enter_context(tc_.tile_pool(name="vtp", bufs=4))
        otp = ctx.enter_context(tc_.tile_pool(name="otp", bufs=4))

        # ---------------- DMA in ----------------
        xsb = [xp.tile([P, S], BF16, name="xsb") for _ in range(ET)]
        wqsb = [wqp.tile([P, JQ], BF16, name="wqsb") for _ in range(ET)]
        wkvsb = [wkvp.tile([P, 2 * JQ], BF16, name="wkvsb") for _ in range(ET)]
        wosb = [wop.tile([P, E], BF16, name="wosb") for _ in range(NPAIR)]
        # first two tiles via GpSimd: its queue is empty at t=0, so these
        # transfers start ~3us before Sync's first trigger fires
        nc.gpsimd.dma_start(wqsb[0][:], wq[ts(0, P), :])
        nc.gpsimd.dma_start(xsb[0][:], xT[ts(0, P), :])
        for i in range(1, ET):
            nc.sync.dma_start(wqsb[i][:], wq[ts(i, P), :])
            nc.sync.dma_start(xsb[i][:], xT[ts(i, P), :])
        for i in range(ET):
            nc.sync.dma_start(wkvsb[i][:], wkv[ts(i, P), :])
        for p in range(NPAIR):
            nc.sync.dma_start(wosb[p][:], wo[ts(p, P), :])


        # PE warm-up: dummy matmuls during the DMA head keep the HAM
        # activity monitor busy so real matmuls start at 2.4 GHz
        warm = ctx.enter_context(tc_.tile_pool(name="warm", bufs=1))
        wt = warm.tile([P, P], BF16, name="wt")
        nc.gpsimd.memset(wt[:], 0.0)
        with tc_.tile_pool(name="psW", bufs=1, space="PSUM") as psW:
            wps = psW.tile([P, P], F32, name="wps")
            for _ in range(28):
                nc.tensor.matmul(wps[:], wt[:], wt[:], start=True, stop=True)

        # ---------------- phase A: qT ----------------
        qtsb = [qtp.tile([P, S], BF16, name="qtsb") for _ in range(NPAIR)]
        with tc_.tile_pool(name="psA", bufs=8, space="PSUM") as psA:
            for jq in range(NPAIR):
                pst = [psA.tile([P, TW], F32, name="pst") for _ in range(TC)]
                for ei in range(ET):
                    lhsT = wqsb[ei][:, ts(jq, P)]
                    for c in range(TC):
                        nc.tensor.matmul(
                            pst[c][:], lhsT, xsb[ei][:, ts(c, TW)],
                            start=(ei == 0), stop=(ei == ET - 1),
                        )
                for c in range(TC):
                    nc.any.tensor_copy(out=qtsb[jq][:, ts(c, TW)], in_=pst[c][:])

        # ---------------- phase B: kv + M accumulation ----------------
        msb = [msbp.tile([P, P], BF16, name="msb") for _ in range(NPAIR)]
        for p in range(NPAIR):
            nc.vector.memset(msb[p][:], 0.0)
        with tc_.tile_pool(name="psM", bufs=NPAIR, space="PSUM") as psM:
            psMt = [psM.tile([P, 256], F32, name="psMt") for _ in range(NPAIR)]
            with tc_.tile_pool(name="psB", bufs=2, space="PSUM") as psB:
                for t in range(TT):
                    c, tl = divmod(t, TC)
                    psk = psB.tile([P, TW], F32)
                    psv = psB.tile([P, TW], F32)
                    for ei in range(ET):
                        lhsT = xsb[ei][:, ts(t, P)]
                        nc.tensor.matmul(
                            psk[:], lhsT, wkvsb[ei][:, 0:JQ],
                            start=(ei == 0), stop=(ei == ET - 1),
                        )
                        nc.tensor.matmul(
                            psv[:], lhsT, wkvsb[ei][:, JQ:2 * JQ],
                            start=(ei == 0), stop=(ei == ET - 1),
                        )
                    kv = kvp.tile([P, 2 * JQ], BF16)
                    nc.any.tensor_copy(out=kv[:, 0:JQ], in_=psk[:])
                    nc.any.tensor_copy(out=kv[:, JQ:2 * JQ], in_=psv[:])
                    for p in range(NPAIR):
                        vo = _voff(p)
                        nc.tensor.matmul(
                            psMt[p][:],
                            kv[:, ts(p, P)],
                            kv[:, JQ + vo:JQ + vo + 256],
                            start=(t == 0), stop=(t == TT - 1),
                            skip_group_check=True,
                        )

            # scale + extract the two diagonal 64x64 blocks per pair
            for p in range(NPAIR):
                c0 = P * p - _voff(p)
                c1 = c0 + D
                nc.vector.tensor_scalar_mul(
                    msb[p][0:D, 0:D], psMt[p][0:D, c0:c0 + D], SCALE)
                nc.vector.tensor_scalar_mul(
                    msb[p][D:P, D:P], psMt[p][D:P, c1:c1 + D], SCALE)

        # ---------------- phase C+D: values and output ----------------
        with (
            tc_.tile_pool(name="psV", bufs=2, space="PSUM") as psV,
            tc_.tile_pool(name="psD", bufs=3, space="PSUM") as psD,
        ):
            vts = [vtp.tile([P, NPAIR, TW], BF16, name="vt")
                   for _ in range(TC)]
            for p in range(NPAIR):
                for c in range(TC):
                    psv2 = psV.tile([P, TW], F32)
                    nc.tensor.matmul(
                        psv2[:], msb[p][:],
                        qtsb[p][:, ts(c, TW)],
                        start=True, stop=True,
                    )
                    nc.any.tensor_copy(out=vts[c][:, p, :], in_=psv2[:])
            for c in range(TC):
                vt = vts[c]
                for tl in range(TC):
                    pd0 = psD.tile([P, TW], F32)
                    pd1 = psD.tile([P, TW], F32)
                    for p in range(NPAIR):
                        lhsT = vt[:, p, ts(tl, P)]
                        nc.tensor.matmul(
                            pd0[:], lhsT, wosb[p][:, 0:TW],
                            start=(p == 0), stop=(p == NPAIR - 1),
                        )
                        nc.tensor.matmul(
                            pd1[:], lhsT, wosb[p][:, TW:E],
                            start=(p == 0), stop=(p == NPAIR - 1),
                        )
                    ot = otp.tile([P, E], F32)
                    nc.any.tensor_copy(out=ot[:, 0:TW], in_=pd0[:])
                    nc.any.tensor_copy(out=ot[:, TW:E], in_=pd1[:])
                    nc.sync.dma_start(out[ts(c * TC + tl, P), :], ot[:])


def _build():
    nc = bacc.Bacc("TRN2", target_bir_lowering=False, debug=False,
                   num_devices=NCORES)
    xT = nc.dram_tensor("xT", [E, S], BF16, kind="ExternalInput").ap()
    wq = nc.dram_tensor("wq", [E, JQ], BF16, kind="ExternalInput").ap()
    wkv = nc.dram_tensor("wkv", [E, 2 * JQ], BF16, kind="ExternalInput").ap()
    wo = nc.dram_tensor("wo", [JQ, E], BF16, kind="ExternalInput").ap()
    out = nc.dram_tensor("out", [S, E], F32, kind="ExternalOutput").ap()

    with tile.TileContext(nc) as tc_:
        _emit(tc_, nc, xT, wq, wkv, wo, out)
    nc.compile()
    return nc


def _in_maps(x, w_qkv, w_o):
    import ml_dtypes
    bf = ml_dtypes.bfloat16
    xTs = [np.ascontiguousarray(x[b].T).astype(bf) for b in range(B)]
    whalf = []
    for hh in range(2):
        hs = range(hh * HPC, (hh + 1) * HPC)
        qrows = np.concatenate([np.arange(192 * h, 192 * h + 64) for h in hs])
        krows = qrows + 64
        vrows = qrows + 128
        wq_ = np.ascontiguousarray(w_qkv[qrows].T).astype(bf)
        wkv_ = np.ascontiguousarray(
            w_qkv[np.concatenate([krows, vrows])].T).astype(bf)
        wo_ = np.ascontiguousarray(w_o[:, hh * JQ:(hh + 1) * JQ].T).astype(bf)
        whalf.append((wq_, wkv_, wo_))
    maps = []
    for core in range(NCORES):
        b, hh = divmod(core, 2)
        wq_, wkv_, wo_ = whalf[hh]
        maps.append({"xT": xTs[b], "wq": wq_, "wkv": wkv_, "wo": wo_})
    return maps


def _gather(results):
    full = np.empty((B, S, E), np.float32)
    for b in range(B):
        full[b] = results[2 * b]["out"] + results[2 * b + 1]["out"]
    return full


def _run(x, w_qkv, w_o, trace=False):
    global _MODULE
    x = np.ascontiguousarray(np.asarray(x, dtype=np.float32))
    w_qkv = np.ascontiguousarray(np.asarray(w_qkv, dtype=np.float32))
    w_o = np.ascontiguousarray(np.asarray(w_o, dtype=np.float32))
    if _MODULE is None:
        _MODULE = _build()
    res = run_bass_kernel_spmd(
        _MODULE, _in_maps(x, w_qkv, w_o),
        core_ids=list(range(NCORES)), trace=trace,
    )
    return _gather(res.results), res


def kernel(x, w_qkv, w_o):
    out, _ = _run(x, w_qkv, w_o, trace=False)
    return out


# revision 22
# speedup vs baseline: 1.0345x; 1.0345x over previous
"""Trainium2 Bass kernel for nn_MultiHeadAttention_70549132804637.

Reference computation (note: NO softmax — values use raw scaled logits):
    qkv = x @ w_qkv.T                         (B,S,3E) split per head into q,k,v
    logits = (q @ k^T) * scale                (B,H,S,S)
    values = logits @ v                       (B,H,S,D)
    out    = concat_heads(values) @ w_o.T     (B,S,E)

Because there is no softmax, (q k^T) v == q (k^T v): we compute the tiny
64x64 per-head matrix M = scale * (k^T v) and then values = q @ M, which
drops the attention cost from O(S^2 D) to O(S D^2) per head.

Sharding over 8 cores: core c handles batch b = c//2 and head-half
hh = c%2 (8 of 16 heads).  Each core computes a partial output
(its heads' contribution through w_o); the host sums the two partials
per batch.  All matmul operands are bf16 (PE upconverts to FP22 and
accumulates FP32 in PSUM; bf16 weights get FWL fast weight-loads, which
hides the per-matmul LDWEIGHTS cost that fp32/fp32r would expose).  A
short stream of dummy matmuls during the DMA lead-in keeps the PE's HAM
activity monitor busy so real matmuls start at the full 2.4 GHz clock.

Per-core dataflow (all tokens T=2048, E=1024, 8 local heads, D=64):
  phase A: qT[j, t]  = wq^T-slices (stationary) x xT (moving)     j=512 rows
  phase B: kv[t, j]  = xT-slices (stationary) x wkv (moving)      j=1024 cols
           M_pair    += k_pair^T @ v-window  (PSUM-resident, 16 t-tiles)
  phase C: msb_pair  = blockdiag(scale*M_h0, scale*M_h1) in SBUF
           vT[f, t]  = msb_pair (stationary) x qT (moving)
  phase D: out[t, u] = vT-slices (stationary) x wo (moving), acc over f
"""

from contextlib import ExitStack

import numpy as np

import concourse.mybir as mybir
import concourse.tile as tile
from concourse import bacc
from concourse.bass import ts
from concourse.bass_utils import run_bass_kernel_spmd

F32 = mybir.dt.float32
BF16 = mybir.dt.bfloat16

B, S, E, H = 4, 2048, 1024, 16
D = E // H                 # 64 per-head dim
SCALE = 0.125              # 1/sqrt(D), exact power of two
NCORES = 8
HPC = H // 2               # heads per core = 8
P = 128
ET = E // P                # 8 e-tiles (contraction tiles for projections)
TC = 4                     # token chunks
TW = S // TC               # 512 tokens per chunk
TT = S // P                # 16 token tiles
NPAIR = HPC // 2           # 4 head pairs per core
JQ = HPC * D               # 512 q columns per core

_MODULE = None


def _voff(p):
    # 256-wide window into the v block covering the pair's 128 columns
    return min(128 * p, 256)


def _emit(tc_, nc, xT, wq, wkv, wo, out):
    with ExitStack() as ctx:
        xp = ctx.enter_context(tc_.tile_pool(name="xp", bufs=ET))
        wqp = ctx.enter_context(tc_.tile_pool(name="wqp", bufs=ET))
        wkvp = ctx.enter_context(tc_.tile_pool(name="wkvp", bufs=ET))
        wop = ctx.enter_context(tc_.tile_pool(name="wop", bufs=NPAIR))
        qtp = ctx.enter_context(tc_.tile_pool(name="qtp", bufs=NPAIR))
        kvp = ctx.enter_context(tc_.tile_pool(name="kvp", bufs=4))
        msbp = ctx.enter_context(tc_.tile_pool(name="msbp", bufs=NPAIR))
        vtp = ctx.enter_context(tc_.tile_pool(name="vtp", bufs=4))
        otp = ctx.enter_context(tc_.tile_pool(name="otp", bufs=4))

        # ---------------- DMA in ----------------
        xsb = [xp.tile([P, S], BF16, name="xsb") for _ in range(ET)]
        wqsb = [wqp.tile([P, JQ], BF16, name="wqsb") for _ in range(ET)]
        wkvsb = [wkvp.tile([P, 2 * JQ], BF16, name="wkvsb") for _ in range(ET)]
        wosb = [wop.tile([P, E], BF16, name="wosb") for _ in range(NPAIR)]
        for i in range(ET):
            nc.sync.dma_start(wqsb[i][:], wq[ts(i, P), :])
            nc.sync.dma_start(xsb[i][:], xT[ts(i, P), :])
        for i in range(ET):
            nc.sync.dma_start(wkvsb[i][:], wkv[ts(i, P), :])
        for p in range(NPAIR):
            nc.sync.dma_start(wosb[p][:], wo[ts(p, P), :])


        # PE warm-up: dummy matmuls during the DMA head keep the HAM
        # activity monitor busy so real matmuls start at 2.4 GHz
        warm = ctx.enter_context(tc_.tile_pool(name="warm", bufs=1))
        wt = warm.tile([P, P], BF16, name="wt")
        nc.gpsimd.memset(wt[:], 0.0)
        with tc_.tile_pool(name="psW", bufs=1, space="PSUM") as psW:
            wps = psW.tile([P, P], F32, name="wps")
            for _ in range(44):
                nc.tensor.matmul(wps[:], wt[:], wt[:], start=True, stop=True)

        # ---------------- phase A: qT ----------------
        qtsb = [qtp.tile([P, S], BF16, name="qtsb") for _ in range(NPAIR)]
        with tc_.tile_pool(name="psA", bufs=8, space="PSUM") as psA:
            for jq in range(NPAIR):
                pst = [psA.tile([P, TW], F32, name="pst") for _ in range(TC)]
                for ei in range(ET):
                    lhsT = wqsb[ei][:, ts(jq, P)]
                    for c in range(TC):
                        nc.tensor.matmul(
                            pst[c][:], lhsT, xsb[ei][:, ts(c, TW)],
                            start=(ei == 0), stop=(ei == ET - 1),
                        )
                for c in range(TC):
                    nc.any.tensor_copy(out=qtsb[jq][:, ts(c, TW)], in_=pst[c][:])

        # ---------------- phase B: kv + M accumulation ----------------
        msb = [msbp.tile([P, P], BF16, name="msb") for _ in range(NPAIR)]
        for p in range(NPAIR):
            nc.vector.memset(msb[p][:], 0.0)
        with tc_.tile_pool(name="psM", bufs=NPAIR, space="PSUM") as psM:
            psMt = [psM.tile([P, 256], F32, name="psMt") for _ in range(NPAIR)]
            with tc_.tile_pool(name="psB", bufs=2, space="PSUM") as psB:
                for t in range(TT):
                    c, tl = divmod(t, TC)
                    psk = psB.tile([P, TW], F32)
                    psv = psB.tile([P, TW], F32)
                    for ei in range(ET):
                        lhsT = xsb[ei][:, ts(t, P)]
                        nc.tensor.matmul(
                            psk[:], lhsT, wkvsb[ei][:, 0:JQ],
                            start=(ei == 0), stop=(ei == ET - 1),
                        )
                        nc.tensor.matmul(
                            psv[:], lhsT, wkvsb[ei][:, JQ:2 * JQ],
                            start=(ei == 0), stop=(ei == ET - 1),
                        )
                    kv = kvp.tile([P, 2 * JQ], BF16)
                    nc.any.tensor_copy(out=kv[:, 0:JQ], in_=psk[:])
                    nc.any.tensor_copy(out=kv[:, JQ:2 * JQ], in_=psv[:])
                    for p in range(NPAIR):
                        vo = _voff(p)
                        nc.tensor.matmul(
                            psMt[p][:],
                            kv[:, ts(p, P)],
                            kv[:, JQ + vo:JQ + vo + 256],
                            start=(t == 0), stop=(t == TT - 1),
                            skip_group_check=True,
                        )

            # scale + extract the two diagonal 64x64 blocks per pair
            for p in range(NPAIR):
                c0 = P * p - _voff(p)
                c1 = c0 + D
                nc.vector.tensor_scalar_mul(
                    msb[p][0:D, 0:D], psMt[p][0:D, c0:c0 + D], SCALE)
                nc.vector.tensor_scalar_mul(
                    msb[p][D:P, D:P], psMt[p][D:P, c1:c1 + D], SCALE)

        # ---------------- phase C+D: values and output ----------------
        with (
            tc_.tile_pool(name="psV", bufs=2, space="PSUM") as psV,
            tc_.tile_pool(name="psD", bufs=3, space="PSUM") as psD,
        ):
            vts = [vtp.tile([P, NPAIR, TW], BF16, name="vt")
                   for _ in range(TC)]
            for p in range(NPAIR):
                for c in range(TC):
                    psv2 = psV.tile([P, TW], F32)
                    nc.tensor.matmul(
                        psv2[:], msb[p][:],
                        qtsb[p][:, ts(c, TW)],
                        start=True, stop=True,
                    )
                    nc.any.tensor_copy(out=vts[c][:, p, :], in_=psv2[:])
            for c in range(TC):
                vt = vts[c]
                for tl in range(TC):
                    pd0 = psD.tile([P, TW], F32)
                    pd1 = psD.tile([P, TW], F32)
                    for p in range(NPAIR):
                        lhsT = vt[:, p, ts(tl, P)]
                        nc.tensor.matmul(
                            pd0[:], lhsT, wosb[p][:, 0:TW],
                            start=(p == 0), stop=(p == NPAIR - 1),
                        )
                        nc.tensor.matmul(
                            pd1[:], lhsT, wosb[p][:, TW:E],
                            start=(p == 0), stop=(p == NPAIR - 1),
                        )
                    ot = otp.tile([P, E], F32)
                    nc.any.tensor_copy(out=ot[:, 0:TW], in_=pd0[:])
                    nc.any.tensor_copy(out=ot[:, TW:E], in_=pd1[:])
                    nc.sync.dma_start(out[ts(c * TC + tl, P), :], ot[:])


def _build():
    nc = bacc.Bacc("TRN2", target_bir_lowering=False, debug=False,
                   num_devices=NCORES)
    xT = nc.dram_tensor("xT", [E, S], BF16, kind="ExternalInput").ap()
    wq = nc.dram_tensor("wq", [E, JQ], BF16, kind="ExternalInput").ap()
    wkv = nc.dram_tensor("wkv", [E, 2 * JQ], BF16, kind="ExternalInput").ap()
    wo = nc.dram_tensor("wo", [JQ, E], BF16, kind="ExternalInput").ap()
    out = nc.dram_tensor("out", [S, E], F32, kind="ExternalOutput").ap()

    with tile.TileContext(nc) as tc_:
        _emit(tc_, nc, xT, wq, wkv, wo, out)
    nc.compile()
    return nc


def _in_maps(x, w_qkv, w_o):
    import ml_dtypes
    bf = ml_dtypes.bfloat16
    xTs = [np.ascontiguousarray(x[b].T).astype(bf) for b in range(B)]
    whalf = []
    for hh in range(2):
        hs = range(hh * HPC, (hh + 1) * HPC)
        qrows = np.concatenate([np.arange(192 * h, 192 * h + 64) for h in hs])
        krows = qrows + 64
        vrows = qrows + 128
        wq_ = np.ascontiguousarray(w_qkv[qrows].T).astype(bf)
        wkv_ = np.ascontiguousarray(
            w_qkv[np.concatenate([krows, vrows])].T).astype(bf)
        wo_ = np.ascontiguousarray(w_o[:, hh * JQ:(hh + 1) * JQ].T).astype(bf)
        whalf.append((wq_, wkv_, wo_))
    maps = []
    for core in range(NCORES):
        b, hh = divmod(core, 2)
        wq_, wkv_, wo_ = whalf[hh]
        maps.append({"xT": xTs[b], "wq": wq_, "wkv": wkv_, "wo": wo_})
    return maps


def _gather(results):
    full = np.empty((B, S, E), np.float32)
    for b in range(B):
        full[b] = results[2 * b]["out"] + results[2 * b + 1]["out"]
    return full


def _run(x, w_qkv, w_o, trace=False):
    global _MODULE
    x = np.ascontiguousarray(np.asarray(x, dtype=np.float32))
    w_qkv = np.ascontiguousarray(np.asarray(w_qkv, dtype=np.float32))
    w_o = np.ascontiguousarray(np.asarray(w_o, dtype=np.float32))
    if _MODULE is None:
        _MODULE = _build()
    res = run_bass_kernel_spmd(
        _MODULE, _in_maps(x, w_qkv, w_o),
        core_ids=list(range(NCORES)), trace=trace,
    )
    return _gather(res.results), res


def kernel(x, w_qkv, w_o):
    out, _ = _run(x, w_qkv, w_o, trace=False)
    return out
